# revision 40
# baseline (speedup 1.0000x reference)
"""Trainium2 Bass kernel for nn_AdvancedLSTMModel (B=262144, D=512, H=16).

The reference network collapses algebraically:
  - seq_len == 1 with zero initial state => LSTM cell is
      h = sigmoid(o) * tanh(sigmoid(i) * tanh(g)),  gates = x @ W_ih.T + b
    (forget gate f is computed but unused since c0 == 0)
  - softmax over a single timestep == 1, so attention context == h1
  - output = h1 @ fc_w.T + fc_b

Strategy: pure data parallel over 8 NeuronCores (batch sharded 32768 rows
per core). The host pre-transposes each x shard to feature-major layout
[128, 4, 32768] so the device streams x.T tiles directly as the matmul
moving operand (contraction over D on partitions; no on-device transpose).

On-device layout: batch is processed in groups of 2048 rows = 4 blocks of
512. All gate tensors are "block-packed": a [128, 512] tile whose partition
quarter q holds the 32 gate/feature channels of block q. Layer-0 gate
matmuls are 4-way column-tiled (M=32 per block into psum partitions 32q),
layer-1 matmuls are 4-way diagonal-tiled (K=32, M=32 at tile (32q, 32q)),
so ScalarE activations and VectorE multiplies always run on full 128
partitions.
"""

import sys

import numpy as np

import concourse.bass as bass
import concourse.mybir as mybir
import concourse.tile as tile
from concourse.bass_utils import run_bass_kernel_spmd

N_CORES = 8
B, D, H = 262144, 512, 16
RC = B // N_CORES          # rows per core
KCH = D // 128             # 4 contraction chunks
BLK = 512                  # rows per block (matmul moving N / psum bank)
GRP = 4 * BLK              # 2048 rows per group (4 column-tiled blocks)
NGRP = RC // GRP           # 16 groups per core
OUT_DMA_GROUPS = 4         # groups batched per output DMA

# Compute dtype for x / weights / activations (psum accumulation is fp32).
DT_NP = np.float16
DT_MB = mybir.dt.float16
F32 = mybir.dt.float32

AF = mybir.ActivationFunctionType


def _patched_drain_and_barrier(self, tick_clock, wait_clock):
    # The nix walrus only encodes one sync-wait per CTRL (drain) instruction;
    # split the Tile tail-drain's waits across one drain each.
    nc = self.nc
    from concourse.tile import ScopedClock

    drain_inst = nc.sync.drain()
    wait_clock.add_sem_waits(
        drain_inst.ins, ScopedClock({None: tick_clock.global_clock})
    )
    si = drain_inst.ins.sync_info
    if si is not None and si.on_wait and len(si.on_wait) > 1:
        waits = list(si.on_wait)
        si.on_wait = waits[:1]
        for w in waits[1:]:
            d2 = nc.sync.drain()
            d2.ins.sync_info = mybir.SyncInfo(on_wait=[w], on_update=[])
    nc.all_engine_barrier()
    popped = nc._tile_sem_poison_stack.pop()
    assert popped is self._sem_poison
    nc.clear_and_free_semaphores(list(self.sems.allocated().values()))
    nc.all_engine_barrier()


tile.TileContext._drain_and_barrier = _patched_drain_and_barrier

MAX_WAITS_PER_INST = 1


def _split_multi_waits(nc, limit=MAX_WAITS_PER_INST):
    """The nix walrus encodes at most `limit` sync-waits per instruction.
    Hoist excess waits onto preceding same-engine nops (engine queues are
    FIFO, so a nop-carried wait gates the next instruction identically)."""
    f = nc.m.functions[0]
    n_split = 0
    for bb in f.blocks:
        insts = bb.instructions
        out = []
        changed = False
        for inst in insts:
            si = inst.sync_info
            if si is not None and si.on_wait and len(si.on_wait) > limit:
                waits = list(si.on_wait)
                head, tail = waits[:-limit], waits[-limit:]
                for i in range(0, len(head), limit):
                    nop = mybir.InstNoOp(
                        name=nc.get_next_instruction_name(), ins=[], outs=[]
                    )
                    nop.engine = inst.engine
                    nop.sync_info = mybir.SyncInfo(
                        on_wait=head[i:i + limit], on_update=[]
                    )
                    out.append(nop)
                si.on_wait = tail
                changed = True
                n_split += 1
            out.append(inst)
        if changed:
            bb.instructions = out
    return n_split


def build_kernel():
    nc = bass.Bass()
    xt = nc.dram_tensor("xt", [128, KCH, RC], DT_MB, kind="ExternalInput")
    w0 = nc.dram_tensor("w0", [128, 3 * KCH * 128], DT_MB, kind="ExternalInput")
    bmm = nc.dram_tensor("bmm", [128, 512], DT_MB, kind="ExternalInput")
    w1 = nc.dram_tensor("w1", [128, 3 * 128], DT_MB, kind="ExternalInput")
    fcw = nc.dram_tensor("fcw", [128, 4], DT_MB, kind="ExternalInput")
    bias = nc.dram_tensor("bias", [128, 8], F32, kind="ExternalInput")
    out = nc.dram_tensor("out", [4, NGRP * BLK], F32, kind="ExternalOutput")

    with tile.TileContext(nc) as tc:
        with (
            tc.tile_pool(name="const", bufs=1) as cpool,
            tc.tile_pool(name="xin", bufs=2) as xpool,
            tc.tile_pool(name="work", bufs=3) as wpool,
            tc.tile_pool(name="outp", bufs=2) as opool,
            tc.tile_pool(name="ps0", bufs=1, space="PSUM") as psp0,
            tc.tile_pool(name="ps1", bufs=1, space="PSUM") as psp1,
            tc.tile_pool(name="ps_fc", bufs=2, space="PSUM") as ps_fc,
        ):
            # startup order: i-pass weights, then the first group's x chunks,
            # then everything else — so the first matmuls start ASAP
            w0_sb = cpool.tile([128, 3 * KCH * 128], DT_MB)
            nc.sync.dma_start(out=w0_sb[:, 0:KCH * 128], in_=w0[:, 0:KCH * 128])
            xg_first = xpool.tile([128, KCH, 2 * GRP], DT_MB, tag="xg")
            for k in range(KCH):
                nc.sync.dma_start(out=xg_first[:, k, 0:GRP],
                                  in_=xt[:, k, 0:GRP])
            nc.sync.dma_start(out=w0_sb[:, KCH * 128:3 * KCH * 128],
                              in_=w0[:, KCH * 128:3 * KCH * 128])
            bias_sb = cpool.tile([128, 8], F32)
            nc.sync.dma_start(out=bias_sb[:], in_=bias[:])
            bmm_sb = cpool.tile([128, 512], DT_MB)
            nc.sync.dma_start(out=bmm_sb[:], in_=bmm[:])
            w1_sb = cpool.tile([128, 3 * 128], DT_MB)
            nc.sync.dma_start(out=w1_sb[:], in_=w1[:])
            fcw_sb = cpool.tile([128, 4], DT_MB)
            nc.sync.dma_start(out=fcw_sb[:], in_=fcw[:])
            ones_sb = cpool.tile([128, BLK], DT_MB)
            nc.vector.memset(ones_sb[:], 1.0)

            # two-stage software pipeline over groups: stage B (layer 1, fc,
            # out) for group t-1 is emitted before stage A (x DMA, layer 0)
            # for group t, so every engine's FIFO leads with ready work.
            stash = {}
            ob = None

            def stage_a_mm(g):
                if g == 0:
                    xg2 = xg_first
                    nc.sync.dma_start(out=xg2[:, :, GRP:2 * GRP],
                                      in_=xt[:, :, GRP:2 * GRP])
                    stash["xg"] = xg2
                elif g % 2 == 0:
                    xg2 = xpool.tile([128, KCH, 2 * GRP], DT_MB, tag="xg")
                    nc.sync.dma_start(out=xg2[:],
                                      in_=xt[:, :, g * GRP:(g + 2) * GRP])
                    stash["xg"] = xg2
                else:
                    xg2 = stash["xg"]
                xg = xg2[:, :, (g % 2) * GRP:(g % 2 + 1) * GRP]
                # sigma(i)+sigma(o) share one 2-bank psum tile; their biases
                # are added as a 5th accumulation chunk (ones x bias/128) so
                # one wide sigmoid act covers both with zero act-bias.
                pio = psp0.tile([128, 2 * BLK], F32, tag="l0io")
                for half in range(2):  # 0: i, 1: o
                    sl = slice(half * BLK, (half + 1) * BLK)
                    for k in range(KCH):
                        for b in range(4):
                            off = (half * KCH + k) * 128 + 32 * b
                            nc.tensor.matmul(
                                pio[32 * b:32 * b + 32, sl],
                                lhsT=w0_sb[:, off:off + 32],
                                rhs=xg[:, k, BLK * b:BLK * (b + 1)],
                                start=(k == 0),
                                stop=False,
                                tile_position=(0, 32 * b),
                            )
                    for b in range(4):
                        boff = 128 * half + 32 * b
                        nc.tensor.matmul(
                            pio[32 * b:32 * b + 32, sl],
                            lhsT=bmm_sb[:, boff:boff + 32],
                            rhs=ones_sb[:],
                            start=False,
                            stop=True,
                            tile_position=(0, 32 * b),
                        )
                pg = psp0.tile([128, BLK], F32, tag="l0g")
                for k in range(KCH):
                    for b in range(4):
                        off = (2 * KCH + k) * 128 + 32 * b
                        nc.tensor.matmul(
                            pg[32 * b:32 * b + 32, :],
                            lhsT=w0_sb[:, off:off + 32],
                            rhs=xg[:, k, BLK * b:BLK * (b + 1)],
                            start=(k == 0),
                            stop=(k == KCH - 1),
                            tile_position=(0, 32 * b),
                        )
                stash[("ps0", g)] = (pio, pg)

            def stage_b_mm(g):
                h0 = stash.pop(("h0", g))
                # sigma(i1)+sigma(o1) share one 2-bank psum tile; biases via a
                # second accumulating [128,128] ones-matmul (same tile mode).
                pio1 = psp1.tile([128, 2 * BLK], F32, tag="l1io")
                for half in range(2):  # 0: i1, 1: o1
                    sl = slice(half * BLK, (half + 1) * BLK)
                    nc.tensor.matmul(
                        pio1[:, sl],
                        lhsT=w1_sb[:, 128 * half:128 * (half + 1)],
                        rhs=h0[:],
                        start=True,
                        stop=False,
                        tile_position=(0, 0),
                    )
                    nc.tensor.matmul(
                        pio1[:, sl],
                        lhsT=bmm_sb[:, 256 + 128 * half:256 + 128 * (half + 1)],
                        rhs=ones_sb[:],
                        start=False,
                        stop=True,
                        tile_position=(0, 0),
                    )
                pg1 = psp1.tile([128, BLK], F32, tag="l1g")
                nc.tensor.matmul(pg1[:], lhsT=w1_sb[:, 256:384], rhs=h0[:],
                                 start=True, stop=True, tile_position=(0, 0))
                stash[("ps1", g)] = (pio1, pg1)

            def stage_acts(t):
                """A-acts(t) interleaved with B-acts(t-1): every ACT op's
                input is ready by the time the FIFO reaches it."""
                has_a, has_b = t < NGRP, t >= 1
                if has_a:
                    pio, pg = stash.pop(("ps0", t))
                    io0 = wpool.tile([128, 2 * BLK], DT_MB, tag="io0")
                    g0 = wpool.tile([128, BLK], DT_MB, tag="g0")
                    nc.scalar.activation(io0[:], pio[:], AF.Sigmoid,
                                         bias=bias_sb[:, 7:8])
                    nc.scalar.activation(g0[:], pg[:], AF.Tanh,
                                         bias=bias_sb[:, 2:3])
                    nc.vector.tensor_mul(io0[:, 0:BLK], io0[:, 0:BLK], g0[:])
                if has_b:
                    pio1, pg1 = stash.pop(("ps1", t - 1))
                    io1 = wpool.tile([128, 2 * BLK], DT_MB, tag="io1")
                    g1 = wpool.tile([128, BLK], DT_MB, tag="g1")
                    nc.scalar.activation(io1[:], pio1[:], AF.Sigmoid,
                                         bias=bias_sb[:, 7:8])
                if has_a:
                    nc.scalar.activation(g0[:], io0[:, 0:BLK], AF.Tanh,
                                         bias=bias_sb[:, 7:8])
                if has_b:
                    nc.scalar.activation(g1[:], pg1[:], AF.Tanh,
                                         bias=bias_sb[:, 5:6])
                if has_a:
                    h0 = wpool.tile([128, BLK], DT_MB, tag="h0")
                    nc.vector.tensor_mul(h0[:], io0[:, BLK:2 * BLK], g0[:])
                    stash[("h0", t)] = h0
                if has_b:
                    nc.vector.tensor_mul(io1[:, 0:BLK], io1[:, 0:BLK], g1[:])
                    nc.scalar.activation(g1[:], io1[:, 0:BLK], AF.Tanh,
                                         bias=bias_sb[:, 7:8])
                    h1 = wpool.tile([128, BLK], DT_MB, tag="h1")
                    nc.vector.tensor_mul(h1[:], io1[:, BLK:2 * BLK], g1[:])
                    stash[("h1", t - 1)] = h1

            def stage_fc(g):
                nonlocal ob
                h1 = stash.pop(("h1", g))
                pf = ps_fc.tile([4, BLK], F32, tag="fc")
                nc.tensor.matmul(pf[:], lhsT=fcw_sb[:, 0:4], rhs=h1[:],
                                 start=True, stop=True, tile_position=(0, 0))
                if g % OUT_DMA_GROUPS == 0:
                    ob = opool.tile([4, OUT_DMA_GROUPS * BLK], F32, tag="ob")
                go = g % OUT_DMA_GROUPS
                # fc bias-add + psum evacuation on the (idle) vector engine
                nc.vector.tensor_scalar_add(ob[:, go * BLK:(go + 1) * BLK],
                                            pf[:], bias_sb[0:4, 6:7])
                if go == OUT_DMA_GROUPS - 1:
                    j = g // OUT_DMA_GROUPS
                    w = OUT_DMA_GROUPS * BLK
                    nc.sync.dma_start(out=out[:, j * w:(j + 1) * w], in_=ob[:])

            # slot t: B-MM(t-1) | interleaved acts(A t, B t-1) | A-MM(t+1) |
            # fc(t-1) — the fc matmul (gated on the full act chain) sits last
            # in the PE FIFO behind ready work.
            stage_a_mm(0)
            for t in range(0, NGRP + 1):
                if t >= 1:
                    stage_b_mm(t - 1)
                stage_acts(t)
                if t + 1 < NGRP:
                    stage_a_mm(t + 1)
                if t >= 1:
                    stage_fc(t - 1)
    _split_multi_waits(nc)
    return nc


def _prep_shared(wf0, bf0, wb0, bb0, wf1, bf1, wb1, bb1, attn_w, attn_b,
                 fc_w, fc_b):
    """Build the replicated weight/bias arrays in device layout."""
    # torch LSTM gate row order within [4H]: i, f, g, o
    def rows(w, which):
        s = {"i": 0, "g": 2 * H, "o": 3 * H}[which]
        return w[s:s + H]

    # layer 0 stationary: [128(d), 3(pass), KCH, 128(4 x 32 dup)]
    w0_host = np.zeros((128, 3, KCH, 128), np.float32)
    for pi, which in enumerate(("i", "o", "g")):
        wp = np.concatenate([rows(wf0, which), rows(wb0, which)], axis=0)  # [32, D]
        for k in range(KCH):
            blk = wp[:, 128 * k:128 * (k + 1)].T  # [128(d), 32]
            for b in range(4):
                w0_host[:, pi, k, 32 * b:32 * (b + 1)] = blk
    w0_host = w0_host.reshape(128, 3 * KCH * 128).astype(DT_NP)

    # layer 1 stationary: block-diagonal [128, 3*128] (per pass, block b of
    # the contraction maps to output block b)
    w1_host = np.zeros((128, 3 * 128), np.float32)
    for pi, which in enumerate(("i", "o", "g")):
        wp = np.concatenate([rows(wf1, which), rows(wb1, which)], axis=0)  # [32, 32]
        for b in range(4):
            w1_host[32 * b:32 * (b + 1),
                    128 * pi + 32 * b:128 * pi + 32 * (b + 1)] = wp.T
    w1_host = w1_host.astype(DT_NP)

    # fc: block-diagonal [128, 4]
    fcw_host = np.zeros((128, 4), np.float32)
    for b in range(4):
        fcw_host[32 * b:32 * (b + 1), b] = fc_w[0]
    fcw_host = fcw_host.astype(DT_NP)

    def brows(bvf, bvb, which):
        s = {"i": 0, "g": 2 * H, "o": 3 * H}[which]
        return np.concatenate([bvf[s:s + H], bvb[s:s + H]])

    bias_host = np.zeros((128, 8), np.float32)
    for col, (bvf, bvb, which) in enumerate((
        (bf0, bb0, "i"), (bf0, bb0, "o"), (bf0, bb0, "g"),
        (bf1, bb1, "i"), (bf1, bb1, "o"), (bf1, bb1, "g"),
    )):
        bias_host[:, col] = np.tile(brows(bvf, bvb, which), 4)
    bias_host[:, 6] = fc_b[0] + attn_b[0] * 0.0  # attn collapses; fc bias only

    # i/o biases routed through a ones-matmul: stationary bias/128
    bmm_host = np.zeros((128, 512), np.float32)
    bmm_host[:, 0:128] = np.tile(brows(bf0, bb0, "i"), 4)[None, :] / 128.0
    bmm_host[:, 128:256] = np.tile(brows(bf0, bb0, "o"), 4)[None, :] / 128.0
    bmm_host[:, 256:384] = np.tile(brows(bf1, bb1, "i"), 4)[None, :] / 128.0
    bmm_host[:, 384:512] = np.tile(brows(bf1, bb1, "o"), 4)[None, :] / 128.0
    bmm_host = bmm_host.astype(DT_NP)
    return w0_host, w1_host, fcw_host, bias_host, bmm_host


_NC_CACHE = None
_LAST_IN_MAPS = None


def last_run_args():
    """For the local test harness: the (in_maps, nc) of the last kernel() call."""
    return _LAST_IN_MAPS, _NC_CACHE


def kernel(**inputs):
    global _NC_CACHE, _LAST_IN_MAPS
    x = np.ascontiguousarray(np.asarray(inputs["x"], dtype=np.float32))
    shared_names = ("wf0", "bf0", "wb0", "bb0", "wf1", "bf1", "wb1", "bb1",
                    "attn_w", "attn_b", "fc_w", "fc_b")
    shared = {k: np.asarray(inputs[k], dtype=np.float32) for k in shared_names}
    w0_host, w1_host, fcw_host, bias_host, bmm_host = _prep_shared(**shared)

    if _NC_CACHE is None:
        _NC_CACHE = build_kernel()
    nc = _NC_CACHE

    in_maps = []
    for c in range(N_CORES):
        xs = x[c * RC:(c + 1) * RC]  # [RC, D]
        # xt[p, k, r] = xs[r, 128k + p]
        xt = xs.reshape(RC, KCH, 128).transpose(2, 1, 0).astype(DT_NP)
        in_maps.append({
            "xt": np.ascontiguousarray(xt),
            "w0": w0_host, "w1": w1_host, "fcw": fcw_host, "bias": bias_host,
            "bmm": bmm_host,
        })

    _LAST_IN_MAPS = in_maps
    try:
        res = run_bass_kernel_spmd(nc, in_maps, core_ids=list(range(N_CORES)))
    except Exception:
        # transient device hiccups (e.g. NRT exec-unit unrecoverable) clear
        # on retry
        import time as _time
        _time.sleep(10)
        res = run_bass_kernel_spmd(nc, in_maps, core_ids=list(range(N_CORES)))
    parts = []
    for c in range(N_CORES):
        o = res.results[c]["out"]  # [4, NGRP*BLK]
        parts.append(
            o.reshape(4, NGRP, BLK).transpose(1, 0, 2).reshape(RC)
        )
    y = np.concatenate(parts)
    return y.reshape(B, 1).astype(np.float32)


# revision 41
# speedup vs baseline: 1.1146x; 1.1146x over previous
"""Trainium2 Bass kernel for nn_AdvancedLSTMModel (B=262144, D=512, H=16).

The reference network collapses algebraically:
  - seq_len == 1 with zero initial state => LSTM cell is
      h = sigmoid(o) * tanh(sigmoid(i) * tanh(g)),  gates = x @ W_ih.T + b
    (forget gate f is computed but unused since c0 == 0)
  - softmax over a single timestep == 1, so attention context == h1
  - output = h1 @ fc_w.T + fc_b

Strategy: pure data parallel over 8 NeuronCores (batch sharded 32768 rows
per core). The host pre-transposes each x shard to feature-major layout
[128, 4, 32768] so the device streams x.T tiles directly as the matmul
moving operand (contraction over D on partitions; no on-device transpose).

On-device layout: batch is processed in groups of 2048 rows = 4 blocks of
512. All gate tensors are "block-packed": a [128, 512] tile whose partition
quarter q holds the 32 gate/feature channels of block q. Layer-0 gate
matmuls are 4-way column-tiled (M=32 per block into psum partitions 32q),
layer-1 matmuls are 4-way diagonal-tiled (K=32, M=32 at tile (32q, 32q)),
so ScalarE activations and VectorE multiplies always run on full 128
partitions.
"""

import sys

import numpy as np

import concourse.bass as bass
import concourse.mybir as mybir
import concourse.tile as tile
from concourse.bass_utils import run_bass_kernel_spmd

N_CORES = 8
B, D, H = 262144, 512, 16
RC = B // N_CORES          # rows per core
KCH = D // 128             # 4 contraction chunks
BLK = 512                  # rows per block (matmul moving N / psum bank)
GRP = 4 * BLK              # 2048 rows per group (4 column-tiled blocks)
NGRP = RC // GRP           # 16 groups per core
OUT_DMA_GROUPS = 4         # groups batched per output DMA

# Compute dtype for x / weights / activations (psum accumulation is fp32).
DT_NP = np.float16
DT_MB = mybir.dt.float16
F32 = mybir.dt.float32

AF = mybir.ActivationFunctionType


def _patched_drain_and_barrier(self, tick_clock, wait_clock):
    # The nix walrus only encodes one sync-wait per CTRL (drain) instruction;
    # split the Tile tail-drain's waits across one drain each.
    nc = self.nc
    from concourse.tile import ScopedClock

    drain_inst = nc.sync.drain()
    wait_clock.add_sem_waits(
        drain_inst.ins, ScopedClock({None: tick_clock.global_clock})
    )
    si = drain_inst.ins.sync_info
    if si is not None and si.on_wait and len(si.on_wait) > 1:
        waits = list(si.on_wait)
        si.on_wait = waits[:1]
        for w in waits[1:]:
            d2 = nc.sync.drain()
            d2.ins.sync_info = mybir.SyncInfo(on_wait=[w], on_update=[])
    nc.all_engine_barrier()
    popped = nc._tile_sem_poison_stack.pop()
    assert popped is self._sem_poison
    nc.clear_and_free_semaphores(list(self.sems.allocated().values()))
    nc.all_engine_barrier()


tile.TileContext._drain_and_barrier = _patched_drain_and_barrier

MAX_WAITS_PER_INST = 1


def _split_multi_waits(nc, limit=MAX_WAITS_PER_INST):
    """The nix walrus encodes at most `limit` sync-waits per instruction.
    Hoist excess waits onto preceding same-engine nops (engine queues are
    FIFO, so a nop-carried wait gates the next instruction identically)."""
    f = nc.m.functions[0]
    n_split = 0
    for bb in f.blocks:
        insts = bb.instructions
        out = []
        changed = False
        for inst in insts:
            si = inst.sync_info
            if si is not None and si.on_wait and len(si.on_wait) > limit:
                waits = list(si.on_wait)
                head, tail = waits[:-limit], waits[-limit:]
                for i in range(0, len(head), limit):
                    nop = mybir.InstNoOp(
                        name=nc.get_next_instruction_name(), ins=[], outs=[]
                    )
                    nop.engine = inst.engine
                    nop.sync_info = mybir.SyncInfo(
                        on_wait=head[i:i + limit], on_update=[]
                    )
                    out.append(nop)
                si.on_wait = tail
                changed = True
                n_split += 1
            out.append(inst)
        if changed:
            bb.instructions = out
    return n_split


def build_kernel():
    nc = bass.Bass()
    xt = nc.dram_tensor("xt", [128, KCH, RC], DT_MB, kind="ExternalInput")
    w0 = nc.dram_tensor("w0", [128, 3 * KCH * 128], DT_MB, kind="ExternalInput")
    bmm = nc.dram_tensor("bmm", [128, 256], DT_MB, kind="ExternalInput")
    w1 = nc.dram_tensor("w1", [128, 3 * 128], DT_MB, kind="ExternalInput")
    fcw = nc.dram_tensor("fcw", [128, 4], DT_MB, kind="ExternalInput")
    bias = nc.dram_tensor("bias", [128, 8], F32, kind="ExternalInput")
    out = nc.dram_tensor("out", [4, NGRP * BLK], F32, kind="ExternalOutput")

    with tile.TileContext(nc) as tc:
        with (
            tc.tile_pool(name="const", bufs=1) as cpool,
            tc.tile_pool(name="xin", bufs=4) as xpool,
            tc.tile_pool(name="work", bufs=3) as wpool,
            tc.tile_pool(name="outp", bufs=2) as opool,
            tc.tile_pool(name="ps0", bufs=1, space="PSUM") as psp0,
            tc.tile_pool(name="ps1", bufs=1, space="PSUM") as psp1,
            tc.tile_pool(name="ps_fc", bufs=2, space="PSUM") as ps_fc,
        ):
            # startup order: i-pass weights, then the first group's x chunks,
            # then everything else — so the first matmuls start ASAP
            w0_sb = cpool.tile([128, 3 * KCH * 128], DT_MB)
            nc.sync.dma_start(out=w0_sb[:, 0:KCH * 128], in_=w0[:, 0:KCH * 128])
            xg_first = xpool.tile([128, KCH, GRP], DT_MB, tag="xg")
            for k in range(KCH):
                nc.sync.dma_start(out=xg_first[:, k, :], in_=xt[:, k, 0:GRP])
            nc.sync.dma_start(out=w0_sb[:, KCH * 128:3 * KCH * 128],
                              in_=w0[:, KCH * 128:3 * KCH * 128])
            bias_sb = cpool.tile([128, 8], F32)
            nc.sync.dma_start(out=bias_sb[:], in_=bias[:])
            bmm_sb = cpool.tile([128, 256], DT_MB)
            nc.sync.dma_start(out=bmm_sb[:], in_=bmm[:])
            w1_sb = cpool.tile([128, 3 * 128], DT_MB)
            nc.sync.dma_start(out=w1_sb[:], in_=w1[:])
            fcw_sb = cpool.tile([128, 4], DT_MB)
            nc.sync.dma_start(out=fcw_sb[:], in_=fcw[:])
            ones_sb = cpool.tile([128, BLK], DT_MB)
            nc.vector.memset(ones_sb[:], 1.0)

            # two-stage software pipeline over groups: stage B (layer 1, fc,
            # out) for group t-1 is emitted before stage A (x DMA, layer 0)
            # for group t, so every engine's FIFO leads with ready work.
            stash = {}
            ob = None

            def stage_a_mm(g):
                if g == 0:
                    xg = xg_first
                else:
                    xg = xpool.tile([128, KCH, GRP], DT_MB, tag="xg")
                    nc.sync.dma_start(out=xg[:],
                                      in_=xt[:, :, g * GRP:(g + 1) * GRP])
                # sigma(i)+sigma(o) share one 2-bank psum tile; their biases
                # are added as a 5th accumulation chunk (ones x bias/128) so
                # one wide sigmoid act covers both with zero act-bias.
                pio = psp0.tile([128, 2 * BLK], F32, tag="l0io")
                for half in range(2):  # 0: i, 1: o
                    sl = slice(half * BLK, (half + 1) * BLK)
                    for k in range(KCH):
                        for b in range(4):
                            off = (half * KCH + k) * 128 + 32 * b
                            nc.tensor.matmul(
                                pio[32 * b:32 * b + 32, sl],
                                lhsT=w0_sb[:, off:off + 32],
                                rhs=xg[:, k, BLK * b:BLK * (b + 1)],
                                start=(k == 0),
                                stop=False,
                                tile_position=(0, 32 * b),
                            )
                    for b in range(4):
                        boff = 128 * half + 32 * b
                        nc.tensor.matmul(
                            pio[32 * b:32 * b + 32, sl],
                            lhsT=bmm_sb[:, boff:boff + 32],
                            rhs=ones_sb[:],
                            start=False,
                            stop=True,
                            tile_position=(0, 32 * b),
                        )
                pg = psp0.tile([128, BLK], F32, tag="l0g")
                for k in range(KCH):
                    for b in range(4):
                        off = (2 * KCH + k) * 128 + 32 * b
                        nc.tensor.matmul(
                            pg[32 * b:32 * b + 32, :],
                            lhsT=w0_sb[:, off:off + 32],
                            rhs=xg[:, k, BLK * b:BLK * (b + 1)],
                            start=(k == 0),
                            stop=(k == KCH - 1),
                            tile_position=(0, 32 * b),
                        )
                stash[("ps0", g)] = (pio, pg)

            def stage_b_mm(g):
                h0 = stash.pop(("h0", g))
                ps = []
                for pi in range(3):
                    # one block-diagonal [128,128] stationary per gate pass
                    p = psp1.tile([128, BLK], F32, tag=f"l1p{pi}")
                    nc.tensor.matmul(
                        p[:],
                        lhsT=w1_sb[:, 128 * pi:128 * (pi + 1)],
                        rhs=h0[:],
                        start=True,
                        stop=True,
                        tile_position=(0, 0),
                    )
                    ps.append(p)
                stash[("ps1", g)] = ps

            def stage_acts(t):
                """A-acts(t) interleaved with B-acts(t-1): every ACT op's
                input is ready by the time the FIFO reaches it."""
                has_a, has_b = t < NGRP, t >= 1
                if has_a:
                    pio, pg = stash.pop(("ps0", t))
                    io0 = wpool.tile([128, 2 * BLK], DT_MB, tag="io0")
                    g0 = wpool.tile([128, BLK], DT_MB, tag="g0")
                    nc.scalar.activation(io0[:], pio[:], AF.Sigmoid,
                                         bias=bias_sb[:, 7:8])
                    nc.scalar.activation(g0[:], pg[:], AF.Tanh,
                                         bias=bias_sb[:, 2:3])
                    nc.vector.tensor_mul(io0[:, 0:BLK], io0[:, 0:BLK], g0[:])
                if has_b:
                    ps1 = stash.pop(("ps1", t - 1))
                    io1 = wpool.tile([128, 2 * BLK], DT_MB, tag="io1")
                    g1 = wpool.tile([128, BLK], DT_MB, tag="g1")
                    nc.scalar.activation(io1[:, 0:BLK], ps1[0][:], AF.Sigmoid,
                                         bias=bias_sb[:, 3:4])
                if has_a:
                    nc.scalar.activation(g0[:], io0[:, 0:BLK], AF.Tanh,
                                         bias=bias_sb[:, 7:8])
                if has_b:
                    nc.scalar.activation(g1[:], ps1[2][:], AF.Tanh,
                                         bias=bias_sb[:, 5:6])
                if has_a:
                    h0 = wpool.tile([128, BLK], DT_MB, tag="h0")
                    nc.vector.tensor_mul(h0[:], io0[:, BLK:2 * BLK], g0[:])
                    stash[("h0", t)] = h0
                if has_b:
                    nc.vector.tensor_mul(io1[:, 0:BLK], io1[:, 0:BLK], g1[:])
                    nc.scalar.activation(io1[:, BLK:2 * BLK], ps1[1][:],
                                         AF.Sigmoid, bias=bias_sb[:, 4:5])
                    nc.scalar.activation(g1[:], io1[:, 0:BLK], AF.Tanh,
                                         bias=bias_sb[:, 7:8])
                    h1 = wpool.tile([128, BLK], DT_MB, tag="h1")
                    nc.vector.tensor_mul(h1[:], io1[:, BLK:2 * BLK], g1[:])
                    stash[("h1", t - 1)] = h1

            def stage_fc(g):
                nonlocal ob
                h1 = stash.pop(("h1", g))
                pf = ps_fc.tile([4, BLK], F32, tag="fc")
                nc.tensor.matmul(pf[:], lhsT=fcw_sb[:, 0:4], rhs=h1[:],
                                 start=True, stop=True, tile_position=(0, 0))
                if g % OUT_DMA_GROUPS == 0:
                    ob = opool.tile([4, OUT_DMA_GROUPS * BLK], F32, tag="ob")
                go = g % OUT_DMA_GROUPS
                # fc bias-add + psum evacuation on the (idle) vector engine
                nc.vector.tensor_scalar_add(ob[:, go * BLK:(go + 1) * BLK],
                                            pf[:], bias_sb[0:4, 6:7])
                if go == OUT_DMA_GROUPS - 1:
                    j = g // OUT_DMA_GROUPS
                    w = OUT_DMA_GROUPS * BLK
                    nc.sync.dma_start(out=out[:, j * w:(j + 1) * w], in_=ob[:])

            # slot t: B-MM(t-1) | interleaved acts(A t, B t-1) | A-MM(t+1) |
            # fc(t-1) — the fc matmul (gated on the full act chain) sits last
            # in the PE FIFO behind ready work.
            stage_a_mm(0)
            for t in range(0, NGRP + 1):
                if t >= 1:
                    stage_b_mm(t - 1)
                stage_acts(t)
                if t + 1 < NGRP:
                    stage_a_mm(t + 1)
                if t >= 1:
                    stage_fc(t - 1)
    _split_multi_waits(nc)
    return nc


def _prep_shared(wf0, bf0, wb0, bb0, wf1, bf1, wb1, bb1, attn_w, attn_b,
                 fc_w, fc_b):
    """Build the replicated weight/bias arrays in device layout."""
    # torch LSTM gate row order within [4H]: i, f, g, o
    def rows(w, which):
        s = {"i": 0, "g": 2 * H, "o": 3 * H}[which]
        return w[s:s + H]

    # layer 0 stationary: [128(d), 3(pass), KCH, 128(4 x 32 dup)]
    w0_host = np.zeros((128, 3, KCH, 128), np.float32)
    for pi, which in enumerate(("i", "o", "g")):
        wp = np.concatenate([rows(wf0, which), rows(wb0, which)], axis=0)  # [32, D]
        for k in range(KCH):
            blk = wp[:, 128 * k:128 * (k + 1)].T  # [128(d), 32]
            for b in range(4):
                w0_host[:, pi, k, 32 * b:32 * (b + 1)] = blk
    w0_host = w0_host.reshape(128, 3 * KCH * 128).astype(DT_NP)

    # layer 1 stationary: block-diagonal [128, 3*128] (per pass, block b of
    # the contraction maps to output block b)
    w1_host = np.zeros((128, 3 * 128), np.float32)
    for pi, which in enumerate(("i", "o", "g")):
        wp = np.concatenate([rows(wf1, which), rows(wb1, which)], axis=0)  # [32, 32]
        for b in range(4):
            w1_host[32 * b:32 * (b + 1),
                    128 * pi + 32 * b:128 * pi + 32 * (b + 1)] = wp.T
    w1_host = w1_host.astype(DT_NP)

    # fc: block-diagonal [128, 4]
    fcw_host = np.zeros((128, 4), np.float32)
    for b in range(4):
        fcw_host[32 * b:32 * (b + 1), b] = fc_w[0]
    fcw_host = fcw_host.astype(DT_NP)

    def brows(bvf, bvb, which):
        s = {"i": 0, "g": 2 * H, "o": 3 * H}[which]
        return np.concatenate([bvf[s:s + H], bvb[s:s + H]])

    bias_host = np.zeros((128, 8), np.float32)
    for col, (bvf, bvb, which) in enumerate((
        (bf0, bb0, "i"), (bf0, bb0, "o"), (bf0, bb0, "g"),
        (bf1, bb1, "i"), (bf1, bb1, "o"), (bf1, bb1, "g"),
    )):
        bias_host[:, col] = np.tile(brows(bvf, bvb, which), 4)
    bias_host[:, 6] = fc_b[0] + attn_b[0] * 0.0  # attn collapses; fc bias only

    # layer-0 i/o biases routed through a ones-matmul: stationary bias/128
    bmm_host = np.zeros((128, 256), np.float32)
    bmm_host[:, 0:128] = np.tile(brows(bf0, bb0, "i"), 4)[None, :] / 128.0
    bmm_host[:, 128:256] = np.tile(brows(bf0, bb0, "o"), 4)[None, :] / 128.0
    bmm_host = bmm_host.astype(DT_NP)
    return w0_host, w1_host, fcw_host, bias_host, bmm_host


_NC_CACHE = None
_LAST_IN_MAPS = None


def last_run_args():
    """For the local test harness: the (in_maps, nc) of the last kernel() call."""
    return _LAST_IN_MAPS, _NC_CACHE


def kernel(**inputs):
    global _NC_CACHE, _LAST_IN_MAPS
    x = np.ascontiguousarray(np.asarray(inputs["x"], dtype=np.float32))
    shared_names = ("wf0", "bf0", "wb0", "bb0", "wf1", "bf1", "wb1", "bb1",
                    "attn_w", "attn_b", "fc_w", "fc_b")
    shared = {k: np.asarray(inputs[k], dtype=np.float32) for k in shared_names}
    w0_host, w1_host, fcw_host, bias_host, bmm_host = _prep_shared(**shared)

    if _NC_CACHE is None:
        _NC_CACHE = build_kernel()
    nc = _NC_CACHE

    in_maps = []
    for c in range(N_CORES):
        xs = x[c * RC:(c + 1) * RC]  # [RC, D]
        # xt[p, k, r] = xs[r, 128k + p]
        xt = xs.reshape(RC, KCH, 128).transpose(2, 1, 0).astype(DT_NP)
        in_maps.append({
            "xt": np.ascontiguousarray(xt),
            "w0": w0_host, "w1": w1_host, "fcw": fcw_host, "bias": bias_host,
            "bmm": bmm_host,
        })

    _LAST_IN_MAPS = in_maps
    try:
        res = run_bass_kernel_spmd(nc, in_maps, core_ids=list(range(N_CORES)))
    except Exception:
        # transient device hiccups (e.g. NRT exec-unit unrecoverable) clear
        # on retry
        import time as _time
        _time.sleep(10)
        res = run_bass_kernel_spmd(nc, in_maps, core_ids=list(range(N_CORES)))
    parts = []
    for c in range(N_CORES):
        o = res.results[c]["out"]  # [4, NGRP*BLK]
        parts.append(
            o.reshape(4, NGRP, BLK).transpose(1, 0, 2).reshape(RC)
        )
    y = np.concatenate(parts)
    return y.reshape(B, 1).astype(np.float32)
